# revision 39
# baseline (speedup 1.0000x reference)
"""2-layer GAT (DGL GATConv) on 8 TRN2 NeuronCores, batch-parallel.

Each core runs one batch element's full graph: N=5000 nodes, E=80000 edges,
128 -> 128 -> 64 features, edge softmax per destination node, final row
softmax.

Structure: nodes are relabeled by in-degree (host-side, free) so each
128-node destination block has near-uniform degree; incoming edges form a
[128 dst-lane x degree] grid (4% slot waste).  A gathered edge slot
(lane d, col j) then lands on partition d, so the per-edge dst attention
term is a per-block broadcast add (no one-hot transpose matmuls), the
softmax denominator is a free-axis reduction, and feature aggregation is
a per-partition pairwise tree-reduce on the DVE -- the tensor engine and
both one-hot matrices (42MB of HBM streams in the naive scheme) vanish
from the edge phase entirely.  W2 folds into the layer-1 gather table
(rows [u = z1@W2aug (66) | el1] = 256B, the dma_gather minimum), so the
L1 epilogue emits layer-2's row table directly and there is no L2 node
phase.  All device work runs in permuted node space; the host unpermutes
the output rows at the end.
"""

import os
import sys
import numpy as np

sys.path.insert(0, "/opt/trn_rl_repo")

import ml_dtypes

import concourse.bass as bass
import concourse.mybir as mybir
from concourse import bacc, tile
from concourse.bass_utils import run_bass_kernel_spmd

BF16 = ml_dtypes.bfloat16

N_NODES = 5000
N_EDGES = 80000
IN_DIM = 128
HID_DIM = 128
OUT_DIM = 64
BATCH = 8
NEG_SLOPE = 0.2
NB = (N_NODES + 127) // 128          # 40 node blocks
NPAD = NB * 128                      # 5120
SB_BLOCKS = 2                        # node blocks per superblock
RW = 128                             # bf16 row width for both gathers (256B)
F1 = OUT_DIM + 2                     # 66: aggregated cols in layer 1 (u)
F2 = OUT_DIM                         # 64: aggregated cols in layer 2

_CACHE = {}


# ----------------------------------------------------------------------------
# Host-side graph preprocessing (pure index manipulation)
# ----------------------------------------------------------------------------

def _host_arrays(src, dst):
    src = np.asarray(src).astype(np.int64).ravel()
    dst = np.asarray(dst).astype(np.int64).ravel()
    deg = np.bincount(dst, minlength=N_NODES)
    order = np.argsort(-deg, kind="stable")        # rank -> original id
    rank = np.empty(N_NODES, np.int64)
    rank[order] = np.arange(N_NODES)
    src_r = rank[src]
    dst_r = rank[dst]

    perm_e = np.argsort(dst_r, kind="stable")
    s_sorted = src_r[perm_e]
    counts = np.bincount(dst_r[perm_e], minlength=NPAD)
    starts = np.concatenate([[0], np.cumsum(counts)])

    chunks_src = []
    chunks_msk = []
    blk_of_chunk = []
    sbs = []
    for b in range(NB):
        lanes = np.arange(b * 128, (b + 1) * 128)
        degs = counts[lanes]
        D = max(1, int(degs.max()))
        for j in range(D):
            srcs = np.zeros(128, np.int64)
            msk = np.zeros(128, np.float32)
            valid = degs > j
            srcs[valid] = s_sorted[starts[lanes[valid]] + j]
            msk[valid] = 1.0
            chunks_src.append(srcs)
            chunks_msk.append(msk)
            blk_of_chunk.append(b)
    blk_of_chunk = np.asarray(blk_of_chunk, np.int64)
    G = len(chunks_src)
    src_pad = np.concatenate(chunks_src)           # slot (c,lane) at c*128+lane

    # dma_gather index layout: unwrapped i = s*16 + (p%16), replicated per core
    E = len(src_pad)
    gidx = np.empty((128, E // 16), np.int16)
    for p16 in range(16):
        gidx[p16, :] = src_pad[p16::16]
    for c in range(1, 8):
        gidx[c * 16:(c + 1) * 16, :] = gidx[:16, :]

    mask_tab = np.stack(chunks_msk, axis=1).astype(BF16)   # [128, G]

    # superblock chunk ranges (SB_BLOCKS node blocks each)
    for b0 in range(0, NB, SB_BLOCKS):
        b1 = min(b0 + SB_BLOCKS, NB)
        chunks = np.nonzero((blk_of_chunk >= b0) & (blk_of_chunk < b1))[0]
        c0, c1 = int(chunks[0]), int(chunks[-1]) + 1
        blks = []
        for b in range(b0, b1):
            bc = np.nonzero(blk_of_chunk == b)[0]
            blks.append((b, int(bc[0]), int(bc[-1]) + 1))
        sbs.append((c0, c1, blks))
    maxch = max(c1 - c0 for c0, c1, _ in sbs)

    return {
        "G": G,
        "gidx": gidx,
        "mask": mask_tab,
        "order": order,
        "sbs": sbs,
        "maxch": maxch,
    }


# ----------------------------------------------------------------------------
# Device kernel builder
# ----------------------------------------------------------------------------

def _build_nc(G, sbs, maxch):
    f32 = mybir.dt.float32
    bf16 = mybir.dt.bfloat16
    i16 = mybir.dt.int16
    AF = mybir.ActivationFunctionType
    ALU = mybir.AluOpType

    nc = bacc.Bacc("TRN2", target_bir_lowering=False, debug=False,
                   num_swdge_queues=4)

    # inputs (Waug1 = [W1@W2aug | W1@al1 | W1@ar1] precomputed on host)
    xT_d = nc.dram_tensor("xT", [128, NPAD], bf16, kind="ExternalInput")
    Waug1_d = nc.dram_tensor("Waug1", [128, F1 + 2], bf16,
                             kind="ExternalInput")
    b2_d = nc.dram_tensor("b2t", [128, OUT_DIM], f32, kind="ExternalInput")
    caug_d = nc.dram_tensor("caug", [128, OUT_DIM], f32, kind="ExternalInput")
    cattn_d = nc.dram_tensor("cattn", [128, 2], f32, kind="ExternalInput")
    mask_d = nc.dram_tensor("mask", [128, G], bf16, kind="ExternalInput")
    gidx_d = nc.dram_tensor("gidx", [128, G * 8], i16, kind="ExternalInput")

    out_d = nc.dram_tensor("out", [N_NODES, OUT_DIM], f32, kind="ExternalOutput")

    # DRAM scratch (gathered-row tables, rank order)
    z1_d = nc.dram_tensor("z1rows", [NPAD, RW], bf16)
    z2_d = nc.dram_tensor("z2rows", [NPAD, RW], bf16)

    with tile.TileContext(nc) as tc:
        # --------------------------------------------------------------
        # persistent SBUF
        # --------------------------------------------------------------
        const = tc.alloc_tile_pool(name="const", bufs=1)
        xT = const.tile([128, NPAD], bf16, tag="xT")
        Waug1 = const.tile([128, F1 + 2], bf16, tag="Waug1")
        b2t = const.tile([128, OUT_DIM], f32, tag="b2t")
        caug = const.tile([128, OUT_DIM], f32, tag="caug")
        cattn = const.tile([128, 2], f32, tag="cattn")
        maskt = const.tile([128, G], bf16, tag="maskt")
        gidx = const.tile([128, G * 8], i16, tag="gidx")
        ercol1 = const.tile([128, NB], bf16, tag="ercol1")
        ercol2 = const.tile([128, NB], bf16, tag="ercol2")

        # strip-split the x load so node-phase matmuls start early
        for s in range(4):
            nc.sync.dma_start(
                out=xT[:, s * (NPAD // 4):(s + 1) * (NPAD // 4)],
                in_=xT_d[:, s * (NPAD // 4):(s + 1) * (NPAD // 4)],
            )
        nc.sync.dma_start(out=Waug1[:, :], in_=Waug1_d[:, :])
        nc.sync.dma_start(out=b2t[:, :], in_=b2_d[:, :])
        nc.sync.dma_start(out=caug[:, :], in_=caug_d[:, :])
        nc.sync.dma_start(out=cattn[:, :], in_=cattn_d[:, :])
        nc.sync.dma_start(out=maskt[:, :], in_=mask_d[:, :])
        nc.sync.dma_start(out=gidx[:, :], in_=gidx_d[:, :])

        # row-assembly pool persists across both layers so layer-1's
        # epilogue (which writes layer-2's rows) shares it
        npool = tc.alloc_tile_pool(name="nprow", bufs=3)

        # ---- L1 node phase: rows [u(66) | el] + er column table ----
        with tc.tile_pool(name="npps", bufs=4, space="PSUM") as npsum:
            for b in range(NB):
                pz = npsum.tile([128, F1 + 2], f32, tag="z")
                nc.tensor.matmul(
                    pz[:, :], xT[:, b * 128:(b + 1) * 128], Waug1[:, :]
                )
                row = npool.tile([128, RW], bf16, tag="row")
                nc.scalar.copy(row[:, 0:F1 + 1], pz[:, 0:F1 + 1])
                nc.vector.tensor_copy(ercol1[:, b:b + 1],
                                      pz[:, F1 + 1:F1 + 2])
                nc.sync.dma_start(
                    out=z1_d[b * 128:(b + 1) * 128, :], in_=row[:, :]
                )

        # --------------------------------------------------------------
        # edge phase: per-lane grid, no one-hots, no tensor engine
        # --------------------------------------------------------------
        def edge_phase(lidx, F, ercol, z_d):
            last = lidx == 2
            elc = F    # el column in the gathered row
            with tc.tile_pool(name=f"zg{lidx}", bufs=4) as zgp, \
                 tc.tile_pool(name=f"ed{lidx}", bufs=3) as edp, \
                 tc.tile_pool(name=f"ep{lidx}", bufs=2) as epp:
                for k, (c0, c1, blks) in enumerate(sbs):
                    nch = c1 - c0
                    zg = zgp.tile([128, maxch, RW], bf16, tag="zg")
                    # split desc-gen across the 4 SWDGE queues (each has
                    # its own descriptor ring)
                    splits = [c0 + (nch * i) // 4 for i in range(4)] + [c1]
                    for q in range(4):
                        q0, q1 = splits[q], splits[q + 1]
                        if q1 == q0:
                            continue
                        ne = (q1 - q0) * 128
                        nc.gpsimd.dma_gather(
                            zg[:, q0 - c0:q1 - c0, :],
                            z_d[:, :],
                            gidx[:, q0 * 8:q1 * 8],
                            ne,
                            ne,
                            RW,
                            single_packet=False,
                            queue_num=q,
                        )
                    # e = el + er[dst-lane] (per-block broadcast); lrelu
                    ee = edp.tile([128, maxch], f32, tag="ee")
                    for (b, bc0, bc1) in blks:
                        nc.vector.tensor_tensor(
                            ee[:, bc0 - c0:bc1 - c0],
                            zg[:, bc0 - c0:bc1 - c0, elc],
                            ercol[:, b:b + 1].to_broadcast([128, bc1 - bc0]),
                            ALU.add,
                        )
                    lr = edp.tile([128, maxch], f32, tag="lr")
                    nc.scalar.mul(lr[:, 0:nch], ee[:, 0:nch], NEG_SLOPE)
                    nc.vector.tensor_tensor(
                        lr[:, 0:nch], lr[:, 0:nch], ee[:, 0:nch], ALU.max
                    )
                    ex = edp.tile([128, maxch], f32, tag="ex")
                    nc.scalar.activation(ex[:, 0:nch], lr[:, 0:nch], AF.Exp)
                    # mask invalid grid slots, then scale gathered rows
                    exm = edp.tile([128, maxch, 1], f32, tag="exm")
                    nc.vector.tensor_tensor(
                        exm[:, 0:nch, 0], ex[:, 0:nch],
                        maskt[:, c0:c1], ALU.mult
                    )
                    nc.vector.tensor_tensor(
                        zg[:, 0:nch, 0:F],
                        zg[:, 0:nch, 0:F],
                        exm[:, 0:nch, :].to_broadcast([128, nch, F]),
                        ALU.mult,
                    )
                    # per block: denominator (free-axis reduce) and
                    # feature aggregation (pairwise tree along columns)
                    for (b, bc0, bc1) in blks:
                        o = bc0 - c0
                        Dfull = bc1 - bc0
                        den = edp.tile([128, 1], f32, tag="den")
                        nc.vector.tensor_reduce(
                            den[:, :], exm[:, o:o + Dfull, 0],
                            axis=mybir.AxisListType.X, op=ALU.add,
                        )
                        D = Dfull
                        while D > 1:
                            h = (D + 1) // 2
                            nc.vector.tensor_tensor(
                                zg[:, o:o + D - h, 0:F],
                                zg[:, o:o + D - h, 0:F],
                                zg[:, o + h:o + D, 0:F],
                                ALU.add,
                            )
                            D = h
                        rec = epp.tile([128, 1], f32, tag="rec")
                        nc.vector.reciprocal(rec[:, :], den[:, :])
                        t = epp.tile([128, F1], f32, tag="t")
                        nc.scalar.mul(t[:, 0:F], zg[:, o, 0:F], rec[:, :])
                        if not last:
                            # z2aug = u_agg/denom + b1@W2aug; emit L2 rows
                            # [z2 | el2] and the er2 column directly
                            row2 = npool.tile([128, RW], bf16, tag="row")
                            nc.vector.tensor_tensor(
                                row2[:, 0:F2], t[:, 0:F2], caug[:, :], ALU.add
                            )
                            nc.vector.tensor_scalar_add(
                                row2[:, F2:F2 + 1], t[:, F2:F2 + 1],
                                cattn[0:128, 0:1],
                            )
                            nc.vector.tensor_scalar_add(
                                ercol2[:, b:b + 1], t[:, F2 + 1:F2 + 2],
                                cattn[0:128, 1:2],
                            )
                            nc.sync.dma_start(
                                out=z2_d[b * 128:(b + 1) * 128, :],
                                in_=row2[:, :],
                            )
                        else:
                            osb = epp.tile([128, OUT_DIM], f32, tag="osb")
                            nc.vector.tensor_tensor(
                                osb[:, :], t[:, 0:F2], b2t[:, :], ALU.add
                            )
                            mx = epp.tile([128, 1], f32, tag="mx")
                            nc.vector.tensor_reduce(
                                mx[:, :], osb[:, :],
                                axis=mybir.AxisListType.X,
                                op=ALU.max, negate=True,
                            )
                            eo = epp.tile([128, OUT_DIM], f32, tag="eo")
                            sden = epp.tile([128, 1], f32, tag="sden")
                            nc.scalar.activation(
                                eo[:, :], osb[:, :], AF.Exp,
                                bias=mx[:, :], accum_out=sden[:, :],
                            )
                            rec2 = epp.tile([128, 1], f32, tag="rec2")
                            nc.vector.reciprocal(rec2[:, :], sden[:, :])
                            ofin = epp.tile([128, OUT_DIM], f32, tag="ofin")
                            nc.scalar.mul(ofin[:, :], eo[:, :], rec2[:, :])
                            nc.sync.dma_start(
                                out=out_d[b * 128:(b + 1) * 128, :]
                                if (b + 1) * 128 <= N_NODES else
                                out_d[b * 128:N_NODES, :],
                                in_=ofin[:, :]
                                if (b + 1) * 128 <= N_NODES else
                                ofin[0:N_NODES - b * 128, :],
                            )

        edge_phase(1, F1, ercol1, z1_d)
        edge_phase(2, F2, ercol2, z2_d)
        npool.release()
        const.release()

    nc.compile()
    return nc


# ----------------------------------------------------------------------------
# entry point
# ----------------------------------------------------------------------------

def _get_compiled(src, dst):
    key = (hash(np.asarray(src).tobytes()), hash(np.asarray(dst).tobytes()))
    if key not in _CACHE:
        host = _host_arrays(src, dst)
        nc = _build_nc(host["G"], host["sbs"], host["maxch"])
        _CACHE[key] = (host, nc)
    return _CACHE[key]


def _make_in_maps(x, W1, al1, ar1, b1, W2, al2, ar2, b2, src, dst):
    host, nc = _get_compiled(src, dst)
    W1f = np.asarray(W1, np.float32)
    W2f = np.asarray(W2, np.float32)
    al1f = np.asarray(al1, np.float32).ravel()
    ar1f = np.asarray(ar1, np.float32).ravel()
    al2f = np.asarray(al2, np.float32).ravel()
    ar2f = np.asarray(ar2, np.float32).ravel()
    b1f = np.asarray(b1, np.float32).ravel()
    b2f = np.asarray(b2, np.float32).ravel()

    # W2 folded into the L1 table: u = z1 @ W2aug (66 cols), since layer 2
    # only consumes these projections of h and aggregation is linear
    W2aug = np.concatenate(
        [W2f, (W2f @ al2f)[:, None], (W2f @ ar2f)[:, None]], 1)  # [128, 66]
    Waug1 = np.concatenate(
        [W1f @ W2aug, (W1f @ al1f)[:, None], (W1f @ ar1f)[:, None]], 1
    ).astype(BF16)                                               # [128, 68]
    caugv = b1f @ W2aug                                          # [66]
    cattn = np.zeros((128, 2), np.float32)
    cattn[:, 0] = caugv[F2]       # c_el2
    cattn[:, 1] = caugv[F2 + 1]   # c_er2

    shared = {
        "Waug1": Waug1,
        "b2t": np.broadcast_to(b2f, (128, OUT_DIM)).copy(),
        "caug": np.broadcast_to(caugv[:F2], (128, OUT_DIM)).copy().astype(
            np.float32),
        "cattn": cattn,
        "mask": host["mask"],
        "gidx": host["gidx"],
    }
    order = host["order"]
    xpad = np.zeros((BATCH, NPAD, IN_DIM), np.float32)
    xpad[:, :N_NODES, :] = np.asarray(x, np.float32)[:, order, :]
    in_maps = [
        {**shared, "xT": np.ascontiguousarray(xpad[b].T).astype(BF16)}
        for b in range(BATCH)
    ]
    return nc, in_maps, order


def kernel(x, W1, al1, ar1, b1, W2, al2, ar2, b2, src, dst):
    nc, in_maps, order = _make_in_maps(x, W1, al1, ar1, b1, W2, al2, ar2, b2,
                                       src, dst)
    res = run_bass_kernel_spmd(nc, in_maps, list(range(BATCH)))
    out = np.empty((BATCH, N_NODES, OUT_DIM), np.float32)
    for b in range(BATCH):
        out[b, order, :] = res.results[b]["out"]     # rank -> original order
    return out.reshape(BATCH * N_NODES, OUT_DIM)


def run_timed(x, W1, al1, ar1, b1, W2, al2, ar2, b2, src, dst, **kw):
    """Run with NTFF profiling; returns exec_time_ns (or None)."""
    nc, in_maps, order = _make_in_maps(x, W1, al1, ar1, b1, W2, al2, ar2, b2,
                                       src, dst)
    res = run_bass_kernel_spmd(nc, in_maps, list(range(BATCH)), trace=True)
    return res.exec_time_ns
